# revision 34
# baseline (speedup 1.0000x reference)
"""BandSplit kernel for 8x Trainium2 NeuronCores.

Math (per batch b, band n, time t):
    feat vector v[64] gathered from (x_real, x_imag) at freqs n*16..n*16+15
    ln  = (v - mean(v)) * rsqrt(var(v)+eps) * ln_w[n] + ln_b[n]
    z[b,n,t,:] = ln @ fc_w[n] + fc_b[n]

Device decomposition (data-parallel over batch, one b per core):
    - Mean-subtraction folded into host-precomputed weights:
        W''[n,f,e] = ln_w[n,f]*fc_w[n,f,e] - (1/64)*sum_g ln_w[n,g]*fc_w[n,g,e]
        b'[n,e]    = fc_b[n,e] + sum_f ln_b[n,f]*fc_w[n,f,e]
      so that  z = rstd * (x @ W'') + b'   exactly.
    - bf16 I/O: x packed to bf16 on host (halves input HBM traffic); z
      stored bf16 [128 e, 2048 (band,t)] per tile (halves output traffic);
      host unpacks to f32 and adds the constant b' during the unpack.
    - Stats: per-band mean / E[x^2] via bf16 ones-matmuls; the 4 tiles of a
      group write 32-aligned strips of a shared [128,512] psum pair per
      t-half, so the variance chain runs batched with no gather DMAs.
    - rstd = exp(-0.5*ln(var+eps)) on the scalar engine: ln/exp/square/copy
      live in one activation table set (Rsqrt/Reciprocal are blocked, the
      custom-DVE reciprocal_approx ops don't lower in this walrus).
    - rstd spread over a pair's 128 partitions by a tiny K=2 matmul into
      psum (gpsimd partition_broadcast hits "ISA wrong length" in codegen;
      partition-step-0 APs are rejected; replicating SBUF->SBUF DMA is ~3us
      of descriptor generation per issue - all measured dead ends).
    - Main matmuls W-stationary: out[e,t] = sum_f W''[f,e] xh[f,t], N=512
      chunks into a 4-deep 1-bank psum ring, drained by scalar ACTIVATE
      Copy (the fastest psum reader: ~2x vector ops or bias-carrying
      activations).
    - ALL DMAs issued from the (otherwise idle) sync engine's HW-DGE queue;
      gpsimd computes x^2 instead of generating DMA descriptors.
    - Software pipeline: stats+rsqrt chain of group g interleaved with
      rb/xh and the main GEMM of group g-1; x loads prefetch one group
      ahead. PSUM: stats 2 + rb 2 + z 4 = 8 banks exactly.
    - Known hardware behavior (measured): matmuls stream 512 cols in ~216ns
      only when weights address, rhs tensor and psum slot deps are all
      unchanged/stale; any weight-address change or semaphore wait adds
      ~300ns. The schedule minimizes both but the per-band weight switch in
      the main GEMM is fundamental.
    - This toolchain allows ONE semaphore wait per instruction; extras are
      hoisted onto NoOps by _legalize_waits.
"""

import numpy as np
import ml_dtypes
from contextlib import ExitStack

import concourse.bass as bass
import concourse.tile as tile
from concourse import mybir
from concourse.bass_utils import run_bass_kernel_spmd

B, C, F, T = 8, 2, 1024, 1024
NB, BW, EMB = 64, 16, 128
FEAT = C * BW * 2  # 64
EPS = 1e-5
NCORES = 8
NTILES = NB // 2   # 32 band-pair tiles per core
GROUP = 4          # tiles per stats batch group (4 x 32-row psum strips)
Nb_GROUPS = NTILES // GROUP

f32 = mybir.dt.float32
bf16 = mybir.dt.bfloat16


def _build_kernel(ctx, tc, xarr, wdup, bsel, zdev):
    nc = tc.nc
    AF = mybir.ActivationFunctionType
    const = ctx.enter_context(tc.tile_pool(name="const", bufs=1))
    xpool = ctx.enter_context(tc.tile_pool(name="xpool", bufs=4 * GROUP))
    sqpool = ctx.enter_context(tc.tile_pool(name="sqpool", bufs=3 * GROUP))
    xhpool = ctx.enter_context(tc.tile_pool(name="xhpool", bufs=GROUP + 2))
    stpool = ctx.enter_context(tc.tile_pool(name="stpool", bufs=2))
    rfpool = ctx.enter_context(tc.tile_pool(name="rfpool", bufs=3))
    zpool = ctx.enter_context(tc.tile_pool(name="zpool", bufs=6))
    # PSUM: stats pm+pe (1 bank each, t-half sequential) + rb 2 + z 4 = 8
    pstat = ctx.enter_context(tc.tile_pool(name="pstat", bufs=1, space="PSUM"))
    prb = ctx.enter_context(tc.tile_pool(name="prb", bufs=2, space="PSUM"))
    pz = ctx.enter_context(tc.tile_pool(name="pz", bufs=4, space="PSUM"))

    # W'' duplicated into both partition halves: [128, NB*EMB] bf16
    W_sb = const.tile([128, NB * EMB], bf16)
    # rstd broadcast selector (host-built): rows 32i/32i+1 spread a band
    # pair's rstd over partitions 0-63 / 64-127
    bsel_sb = const.tile([128, 128], bf16)

    def load_consts():
        nc.sync.dma_start(W_sb[:], wdup[:])
        nc.sync.dma_start(bsel_sb[:], bsel[:])

    # block-diagonal 1/64 selector for per-band mean / E[x^2]; cols 2-31 stay
    # zero so each matmul initializes its whole 32-row psum strip
    onesblk = const.tile([128, 32], bf16)
    nc.vector.memset(onesblk[:], 0.0)
    nc.vector.memset(onesblk[0:64, 0:1], 1.0 / FEAT)
    nc.vector.memset(onesblk[64:128, 1:2], 1.0 / FEAT)
    eps_col = const.tile([128, 1], f32)
    nc.vector.memset(eps_col[:], float(EPS))

    def a_load(g):
        # group 0 squares go on the (startup-idle) vector engine so the PE
        # can start its first stats matmuls sooner
        xs, sqs = [], []
        for i in range(GROUP):
            tj = g * GROUP + i
            xt = xpool.tile([128, T], bf16, tag="xt")
            nc.sync.dma_start(xt[:], xarr[tj])
            sq = sqpool.tile([128, T], bf16, tag="sq")
            eng = nc.vector if g == 0 else nc.gpsimd
            eng.tensor_mul(sq[:], xt[:], xt[:])
            xs.append(xt)
            sqs.append(sq)
        return xs, sqs

    def a_mm(g, st, h):
        # one t-half of the group's stats strips (tile i at psum rows 32i)
        xs, sqs = st
        cs = slice(512 * h, 512 * h + 512)
        P_m = pstat.tile([128, 512], f32, tag="pm", name="pm")
        P_e = pstat.tile([128, 512], f32, tag="pe", name="pe")
        for i in range(GROUP):
            r0 = 32 * i
            nc.tensor.matmul(
                P_m[r0 : r0 + 32, :], onesblk[:], xs[i][:, cs],
                skip_group_check=True, tile_position=(0, r0),
            )
            nc.tensor.matmul(
                P_e[r0 : r0 + 32, :], onesblk[:], sqs[i][:, cs],
                skip_group_check=True, tile_position=(0, r0),
            )
        return P_m, P_e

    def chain_half(g, ph, h, m2):
        # var = E[x^2] - mean^2 for one t-half, freeing the psum pair
        P_m, P_e = ph
        cs = slice(512 * h, 512 * h + 512)
        nc.scalar.activation(m2[:, cs], P_m[:], AF.Square)
        nc.vector.tensor_sub(m2[:, cs], P_e[:], m2[:, cs])

    def chain_end(g, m2):
        # rstd = 1/sqrt(var+eps) = exp(-0.5*ln(var+eps)); ln/exp/square/
        # copy share one scalar act table set (no reloads)
        nc.scalar.activation(m2[:], m2[:], AF.Ln, bias=eps_col[:])
        rb16 = rfpool.tile([128, T], bf16, tag="rb16")
        nc.scalar.activation(rb16[:], m2[:], AF.Exp, scale=-0.5)
        return rb16

    def b_scale(g, xs, rb16):
        # xh = x * rstd; rstd spread over the pair's 128 partitions by a
        # tiny K=2 matmul into psum, consumed right away by the vector mul
        xhs = []
        for i in range(GROUP):
            r0 = 32 * i
            xh = xhpool.tile([128, T], bf16, tag="xh")
            for h in range(2):
                cs = slice(512 * h, 512 * h + 512)
                rbp = prb.tile([128, 512], f32, tag="rb")
                nc.tensor.matmul(
                    rbp[:],
                    bsel_sb[r0 : r0 + 2, :],
                    rb16[r0 : r0 + 2, cs],
                    tile_position=(r0, 0),
                )
                nc.vector.tensor_mul(xh[:, cs], xs[i][:, cs], rbp[:])
            xhs.append(xh)
        return xhs

    def b_mm(g, xhs):
        # main GEMM out[e,t]; fast scalar Copy drains (bias added on host)
        for i in range(GROUP):
            tj = g * GROUP + i
            xh = xhs[i]
            zst = zpool.tile([128, 2 * T], bf16, tag="zst")
            for h in range(2):
                for bl in range(2):
                    n = 2 * tj + bl
                    p0 = 64 * bl
                    pzt = pz.tile([128, 512], f32, tag="pz")
                    nc.tensor.matmul(
                        pzt[:],
                        W_sb[p0 : p0 + 64, n * EMB : (n + 1) * EMB],
                        xh[p0 : p0 + 64, 512 * h : 512 * h + 512],
                        tile_position=(p0, 0),
                    )
                    c0 = bl * T + h * 512
                    nc.scalar.activation(
                        zst[:, c0 : c0 + 512], pzt[:], AF.Copy
                    )
            nc.sync.dma_start(zdev[tj], zst[:])

    # software pipeline: stats/chain(g) interleaved with rb/xh and GEMM of
    # the previous group
    st = {}
    st[0] = a_load(0)
    if Nb_GROUPS > 1:
        st[1] = a_load(1)
    # consts go out after both startup groups' x loads: W isn't needed until
    # the first b_mm a full iteration later, and 2 MB ahead of group 1's x
    # tiles costs the PE its second-iteration head start
    load_consts()
    rb = {}

    def a_and_chain(g):
        m2 = stpool.tile([128, T], f32, tag="m2")
        ph = a_mm(g, st[g], 0)
        yield
        chain_half(g, ph, 0, m2)
        ph = a_mm(g, st[g], 1)
        chain_half(g, ph, 1, m2)
        rb[g] = chain_end(g, m2)

    gen = a_and_chain(0)
    next(gen)
    for _ in gen:
        pass
    if Nb_GROUPS > 2:
        st[2] = a_load(2)
    for g in range(1, Nb_GROUPS):
        if g + 2 < Nb_GROUPS:
            st[g + 2] = a_load(g + 2)
        gen = a_and_chain(g)
        next(gen)                      # A(g) h0 strips emitted
        xhs = b_scale(g - 1, st[g - 1][0], rb[g - 1])
        for _ in gen:                  # C0, A h1, C1
            pass
        b_mm(g - 1, xhs)
        del st[g - 1], rb[g - 1]
    g = Nb_GROUPS - 1
    xhs = b_scale(g, st[g][0], rb[g])
    b_mm(g, xhs)


def _legalize_waits(nc):
    """walrus here accepts ONE sync-wait per instruction; hoist extras onto
    single-wait NoOps inserted just before (same engine, same semantics)."""
    n_split = 0
    for f in nc.m.functions:
        for blk in f.blocks:
            newlist = []
            for ins in blk.instructions:
                si = ins.sync_info
                if si is not None and len(si.on_wait) > 1:
                    waits = list(si.on_wait)
                    for w in waits[:-1]:
                        nop = mybir.InstEventSemaphore(
                            name=f"{ins.name}-w{n_split}",
                            ins=[],
                            outs=[],
                            engine=ins.engine,
                        )
                        nop.sync_info = mybir.SyncInfo(on_wait=[w], on_update=[])
                        newlist.append(nop)
                        n_split += 1
                    ins.sync_info = mybir.SyncInfo(
                        on_wait=[waits[-1]], on_update=list(si.on_update)
                    )
                newlist.append(ins)
            blk.instructions = newlist
    return n_split


def build_nc(legalize=True):
    nc = bass.Bass("TRN2", target_bir_lowering=False, debug=False)
    xarr = nc.dram_tensor("xarr", [NTILES, 128, T], bf16, kind="ExternalInput")
    wdup = nc.dram_tensor("wdup", [128, NB * EMB], bf16, kind="ExternalInput")
    bsel = nc.dram_tensor("bsel", [128, 128], bf16, kind="ExternalInput")
    zdev = nc.dram_tensor("zdev", [NTILES, 128, 2 * T], bf16, kind="ExternalOutput")
    with tile.TileContext(nc) as tc, ExitStack() as ctx:
        _build_kernel(ctx, tc, xarr.ap(), wdup.ap(), bsel.ap(), zdev.ap())
    if legalize:
        _legalize_waits(nc)
    return nc


_NC = None


def _get_nc():
    global _NC
    if _NC is None:
        _NC = build_nc()
    return _NC


def _install_ntff_shim():
    """Register the axon NTFF profile hook (dev/testing only; the image's
    antenv package lacks axon_hooks, so bass_utils trace=True would fail)."""
    import sys
    import types

    if "antenv.axon_hooks" in sys.modules:
        return
    mod = types.ModuleType("antenv.axon_hooks")
    mod._hook = None

    def set_axon_ntff_profile_hook(h):
        mod._hook = h

    def get_axon_ntff_profile_hook():
        return mod._hook

    mod.set_axon_ntff_profile_hook = set_axon_ntff_profile_hook
    mod.get_axon_ntff_profile_hook = get_axon_ntff_profile_hook
    sys.modules["antenv.axon_hooks"] = mod
    try:
        import antenv

        antenv.axon_hooks = mod
    except ImportError:
        pass
    try:
        from trn_agent_boot.trn_boot import _ntff_profile_via_ctypes

        mod._hook = _ntff_profile_via_ctypes("/opt/axon/libaxon_pjrt.so")
    except Exception as e:
        print(f"ntff shim: no hook ({e})")


def _host_params(ln_w, ln_b, fc_w, fc_b):
    lw = ln_w.astype(np.float64)
    lb = ln_b.astype(np.float64)
    fw = fc_w.astype(np.float64)
    fb = fc_b.astype(np.float64)
    W1 = lw[:, :, None] * fw                          # [NB, FEAT, EMB]
    Wpp = W1 - W1.sum(1, keepdims=True) / FEAT        # mean-fold
    bp = fb + (lb[:, :, None] * fw).sum(1)            # [NB, EMB]
    # partition q = i*32 + c*16 + w  maps to feat index c*32 + w*2 + i
    q = np.arange(FEAT)
    i = q // 32
    c = (q % 32) // 16
    w = q % 16
    perm = c * 32 + w * 2 + i
    Wq = Wpp[:, perm, :]                              # [NB, 64(q), EMB]
    whalf = np.ascontiguousarray(Wq.transpose(1, 0, 2).reshape(FEAT, NB * EMB))
    wdup_np = np.concatenate([whalf, whalf], axis=0).astype(ml_dtypes.bfloat16)
    bp_np = bp.astype(np.float32)                     # [NB, EMB]
    bsel_np = np.zeros((128, 128), np.float32)
    for i in range(GROUP):
        bsel_np[32 * i, 0:64] = 1.0
        bsel_np[32 * i + 1, 64:128] = 1.0
    bsel_np = bsel_np.astype(ml_dtypes.bfloat16)
    return wdup_np, bp_np, bsel_np


def _pack_x(x_real, x_imag):
    """[B,C,F,T] x2 -> [B, NTILES, 128, T] bf16: partition q = i*32+c*16+w."""
    xr = x_real.reshape(B, C, NTILES, 2, BW, T)   # [b, c, tile, bp, w, t]
    xi = x_imag.reshape(B, C, NTILES, 2, BW, T)
    out = np.empty((B, NTILES, 2, 2, C, BW, T), np.float32)
    out[:, :, :, 0] = xr.transpose(0, 2, 3, 1, 4, 5)
    out[:, :, :, 1] = xi.transpose(0, 2, 3, 1, 4, 5)
    return np.ascontiguousarray(
        out.reshape(B, NTILES, 128, T).astype(ml_dtypes.bfloat16)
    )


def _unpack_z(zdev, bp):
    """[B, NTILES, 128, 2048] bf16 -> [B, NB, T, EMB] f32 (+ bias b'): per
    tile the free axis is (band, t) and partitions are e."""
    nb = zdev.shape[0]
    z = zdev.astype(np.float32).reshape(nb, NTILES, EMB, 2, T)
    z = z.transpose(0, 1, 3, 4, 2)                # [b, tile, band, t, e]
    z = np.ascontiguousarray(z.reshape(nb, NB, T, EMB))
    z += bp[None, :, None, :]
    return z


def kernel(x_real, x_imag, ln_w, ln_b, fc_w, fc_b, _trace=False):
    x_real = np.asarray(x_real, dtype=np.float32)
    x_imag = np.asarray(x_imag, dtype=np.float32)
    ln_w = np.asarray(ln_w, dtype=np.float32)
    ln_b = np.asarray(ln_b, dtype=np.float32)
    fc_w = np.asarray(fc_w, dtype=np.float32)
    fc_b = np.asarray(fc_b, dtype=np.float32)

    if _trace:
        _install_ntff_shim()
    wdup_np, bp_np, bsel_np = _host_params(ln_w, ln_b, fc_w, fc_b)
    xarr = _pack_x(x_real, x_imag)
    nc = _get_nc()
    in_maps = [
        {
            "xarr": xarr[i],
            "wdup": wdup_np,
            "bsel": bsel_np,
        }
        for i in range(NCORES)
    ]
    res = run_bass_kernel_spmd(nc, in_maps, list(range(NCORES)), trace=_trace)
    if _trace and res.exec_time_ns is not None:
        print(f"HW exec time: {res.exec_time_ns} ns")
        if res.instructions_and_trace is not None:
            print(f"trace: {res.instructions_and_trace[1]}")
    zdev = np.stack([res.results[i]["zdev"] for i in range(NCORES)], axis=0)
    return _unpack_z(zdev, bp_np)


# revision 38
# speedup vs baseline: 1.0090x; 1.0090x over previous
"""BandSplit kernel for 8x Trainium2 NeuronCores.

Math (per batch b, band n, time t):
    feat vector v[64] gathered from (x_real, x_imag) at freqs n*16..n*16+15
    ln  = (v - mean(v)) * rsqrt(var(v)+eps) * ln_w[n] + ln_b[n]
    z[b,n,t,:] = ln @ fc_w[n] + fc_b[n]

Device decomposition (data-parallel over batch, one b per core):
    - Mean-subtraction folded into host-precomputed weights:
        W''[n,f,e] = ln_w[n,f]*fc_w[n,f,e] - (1/64)*sum_g ln_w[n,g]*fc_w[n,g,e]
        b'[n,e]    = fc_b[n,e] + sum_f ln_b[n,f]*fc_w[n,f,e]
      so that  z = rstd * (x @ W'') + b'   exactly.
    - bf16 I/O: x packed to bf16 on host (halves input HBM traffic); z
      stored bf16 [128 e, 2048 (band,t)] per tile (halves output traffic);
      host unpacks to f32 and adds the constant b' during the unpack.
    - Stats: per-band mean / E[x^2] via bf16 ones-matmuls; the 4 tiles of a
      group write 32-aligned strips of a shared [128,512] psum pair per
      t-half, so the variance chain runs batched with no gather DMAs.
    - rstd = exp(-0.5*ln(var+eps)) on the scalar engine: ln/exp/square/copy
      live in one activation table set (Rsqrt/Reciprocal are blocked, the
      custom-DVE reciprocal_approx ops don't lower in this walrus).
    - rstd spread over a pair's 128 partitions by a tiny K=2 matmul into
      psum (gpsimd partition_broadcast hits "ISA wrong length" in codegen;
      partition-step-0 APs are rejected; replicating SBUF->SBUF DMA is ~3us
      of descriptor generation per issue - all measured dead ends).
    - Main matmuls W-stationary: out[e,t] = sum_f W''[f,e] xh[f,t], N=512
      chunks into a 4-deep 1-bank psum ring, drained by scalar ACTIVATE
      Copy (the fastest psum reader: ~2x vector ops or bias-carrying
      activations).
    - ALL DMAs issued from the (otherwise idle) sync engine's HW-DGE queue;
      gpsimd computes x^2 instead of generating DMA descriptors.
    - Software pipeline: stats+rsqrt chain of group g interleaved with
      rb/xh and the main GEMM of group g-1; x loads prefetch one group
      ahead. PSUM: stats 2 + rb 2 + z 4 = 8 banks exactly.
    - Known hardware behavior (measured): matmuls stream 512 cols in ~216ns
      only when weights address, rhs tensor and psum slot deps are all
      unchanged/stale; any weight-address change or semaphore wait adds
      ~300ns. The schedule minimizes both but the per-band weight switch in
      the main GEMM is fundamental.
    - This toolchain allows ONE semaphore wait per instruction; extras are
      hoisted onto NoOps by _legalize_waits.
"""

import numpy as np
import ml_dtypes
from contextlib import ExitStack

import concourse.bass as bass
import concourse.tile as tile
from concourse import mybir
from concourse.bass_utils import run_bass_kernel_spmd

B, C, F, T = 8, 2, 1024, 1024
NB, BW, EMB = 64, 16, 128
FEAT = C * BW * 2  # 64
EPS = 1e-5
NCORES = 8
NTILES = NB // 2   # 32 band-pair tiles per core
GROUP = 4          # tiles per stats batch group (4 x 32-row psum strips)
Nb_GROUPS = NTILES // GROUP

f32 = mybir.dt.float32
bf16 = mybir.dt.bfloat16


def _build_kernel(ctx, tc, xarr, wdup, bsel, zdev):
    nc = tc.nc
    AF = mybir.ActivationFunctionType
    const = ctx.enter_context(tc.tile_pool(name="const", bufs=1))
    xpool = ctx.enter_context(tc.tile_pool(name="xpool", bufs=3 * GROUP))
    sqpool = ctx.enter_context(tc.tile_pool(name="sqpool", bufs=2 * GROUP))
    xhpool = ctx.enter_context(tc.tile_pool(name="xhpool", bufs=2 * GROUP + 1))
    stpool = ctx.enter_context(tc.tile_pool(name="stpool", bufs=2))
    rfpool = ctx.enter_context(tc.tile_pool(name="rfpool", bufs=3))
    zpool = ctx.enter_context(tc.tile_pool(name="zpool", bufs=6))
    # PSUM: stats pm+pe (1 bank each, t-half sequential) + rb 2 + z 4 = 8
    pstat = ctx.enter_context(tc.tile_pool(name="pstat", bufs=1, space="PSUM"))
    prb = ctx.enter_context(tc.tile_pool(name="prb", bufs=2, space="PSUM"))
    pz = ctx.enter_context(tc.tile_pool(name="pz", bufs=4, space="PSUM"))

    # W'' duplicated into both partition halves: [128, NB*EMB] bf16
    W_sb = const.tile([128, NB * EMB], bf16)
    # rstd broadcast selector (host-built): rows 32i/32i+1 spread a band
    # pair's rstd over partitions 0-63 / 64-127
    bsel_sb = const.tile([128, 128], bf16)

    def load_consts():
        nc.sync.dma_start(W_sb[:], wdup[:])
        nc.sync.dma_start(bsel_sb[:], bsel[:])

    # block-diagonal 1/64 selector for per-band mean / E[x^2]; cols 2-31 stay
    # zero so each matmul initializes its whole 32-row psum strip
    onesblk = const.tile([128, 32], bf16)
    nc.vector.memset(onesblk[:], 0.0)
    nc.vector.memset(onesblk[0:64, 0:1], 1.0 / FEAT)
    nc.vector.memset(onesblk[64:128, 1:2], 1.0 / FEAT)
    eps_col = const.tile([128, 1], f32)
    nc.vector.memset(eps_col[:], float(EPS))

    def a_load(g):
        # group 0 squares go on the (startup-idle) vector engine so the PE
        # can start its first stats matmuls sooner
        xs, sqs = [], []
        for i in range(GROUP):
            tj = g * GROUP + i
            xt = xpool.tile([128, T], bf16, tag="xt")
            nc.sync.dma_start(xt[:], xarr[tj])
            sq = sqpool.tile([128, T], bf16, tag="sq")
            eng = nc.vector if g == 0 else nc.gpsimd
            eng.tensor_mul(sq[:], xt[:], xt[:])
            xs.append(xt)
            sqs.append(sq)
        return xs, sqs

    def a_mm(g, st, h):
        # one t-half of the group's stats strips (tile i at psum rows 32i)
        xs, sqs = st
        cs = slice(512 * h, 512 * h + 512)
        P_m = pstat.tile([128, 512], f32, tag="pm", name="pm")
        P_e = pstat.tile([128, 512], f32, tag="pe", name="pe")
        for i in range(GROUP):
            r0 = 32 * i
            nc.tensor.matmul(
                P_m[r0 : r0 + 32, :], onesblk[:], xs[i][:, cs],
                skip_group_check=True, tile_position=(0, r0),
            )
            nc.tensor.matmul(
                P_e[r0 : r0 + 32, :], onesblk[:], sqs[i][:, cs],
                skip_group_check=True, tile_position=(0, r0),
            )
        return P_m, P_e

    def chain_half(g, ph, h, m2):
        # var = E[x^2] - mean^2 for one t-half, freeing the psum pair
        P_m, P_e = ph
        cs = slice(512 * h, 512 * h + 512)
        nc.scalar.activation(m2[:, cs], P_m[:], AF.Square)
        nc.vector.tensor_sub(m2[:, cs], P_e[:], m2[:, cs])

    def chain_end(g, m2):
        # rstd = 1/sqrt(var+eps) = exp(-0.5*ln(var+eps)); ln/exp/square/
        # copy share one scalar act table set (no reloads)
        nc.scalar.activation(m2[:], m2[:], AF.Ln, bias=eps_col[:])
        rb16 = rfpool.tile([128, T], bf16, tag="rb16")
        nc.scalar.activation(rb16[:], m2[:], AF.Exp, scale=-0.5)
        return rb16

    def b_scale(g, xs, rb16):
        # xh = x * rstd; rstd spread over the pair's 128 partitions by a
        # tiny K=2 matmul into psum, consumed right away by the vector mul
        xhs = []
        for i in range(GROUP):
            r0 = 32 * i
            xh = xhpool.tile([128, T], bf16, tag="xh")
            for h in range(2):
                cs = slice(512 * h, 512 * h + 512)
                rbp = prb.tile([128, 512], f32, tag="rb")
                nc.tensor.matmul(
                    rbp[:],
                    bsel_sb[r0 : r0 + 2, :],
                    rb16[r0 : r0 + 2, cs],
                    tile_position=(r0, 0),
                )
                nc.vector.tensor_mul(xh[:, cs], xs[i][:, cs], rbp[:])
            xhs.append(xh)
        return xhs

    def b_mm(g, xhs):
        # main GEMM out[e,t]; fast scalar Copy drains (bias added on host)
        for i in range(GROUP):
            tj = g * GROUP + i
            xh = xhs[i]
            zst = zpool.tile([128, 2 * T], bf16, tag="zst")
            for h in range(2):
                for bl in range(2):
                    n = 2 * tj + bl
                    p0 = 64 * bl
                    pzt = pz.tile([128, 512], f32, tag="pz")
                    nc.tensor.matmul(
                        pzt[:],
                        W_sb[p0 : p0 + 64, n * EMB : (n + 1) * EMB],
                        xh[p0 : p0 + 64, 512 * h : 512 * h + 512],
                        tile_position=(p0, 0),
                    )
                    c0 = bl * T + h * 512
                    nc.scalar.activation(
                        zst[:, c0 : c0 + 512], pzt[:], AF.Copy
                    )
            nc.sync.dma_start(zdev[tj], zst[:])

    # software pipeline: stats/chain(g) interleaved with rb/xh and GEMM of
    # the previous group
    st = {}
    st[0] = a_load(0)
    if Nb_GROUPS > 1:
        st[1] = a_load(1)
    # consts go out after both startup groups' x loads: W isn't needed until
    # the first b_mm a full iteration later, and 2 MB ahead of group 1's x
    # tiles costs the PE its second-iteration head start
    load_consts()
    rb = {}

    def a_and_chain(g):
        m2 = stpool.tile([128, T], f32, tag="m2")
        ph = a_mm(g, st[g], 0)
        yield
        chain_half(g, ph, 0, m2)
        ph = a_mm(g, st[g], 1)
        chain_half(g, ph, 1, m2)
        rb[g] = chain_end(g, m2)

    gen = a_and_chain(0)
    next(gen)
    for _ in gen:
        pass
    for g in range(1, Nb_GROUPS):
        if g + 1 < Nb_GROUPS:
            st[g + 1] = a_load(g + 1)
        gen = a_and_chain(g)
        next(gen)                      # A(g) h0 strips emitted
        xhs = b_scale(g - 1, st[g - 1][0], rb[g - 1])
        for _ in gen:                  # C0, A h1, C1
            pass
        if g == Nb_GROUPS - 1:
            # last group's rb/xh phase before the previous GEMM: its vector
            # muls overlap that GEMM instead of running bare in the epilogue
            xhs_last = b_scale(g, st[g][0], rb[g])
        b_mm(g - 1, xhs)
        del st[g - 1], rb[g - 1]
    b_mm(Nb_GROUPS - 1, xhs_last)


def _legalize_waits(nc):
    """walrus here accepts ONE sync-wait per instruction; hoist extras onto
    single-wait NoOps inserted just before (same engine, same semantics)."""
    n_split = 0
    for f in nc.m.functions:
        for blk in f.blocks:
            newlist = []
            for ins in blk.instructions:
                si = ins.sync_info
                if si is not None and len(si.on_wait) > 1:
                    waits = list(si.on_wait)
                    for w in waits[:-1]:
                        nop = mybir.InstEventSemaphore(
                            name=f"{ins.name}-w{n_split}",
                            ins=[],
                            outs=[],
                            engine=ins.engine,
                        )
                        nop.sync_info = mybir.SyncInfo(on_wait=[w], on_update=[])
                        newlist.append(nop)
                        n_split += 1
                    ins.sync_info = mybir.SyncInfo(
                        on_wait=[waits[-1]], on_update=list(si.on_update)
                    )
                newlist.append(ins)
            blk.instructions = newlist
    return n_split


def build_nc(legalize=True):
    nc = bass.Bass("TRN2", target_bir_lowering=False, debug=False)
    xarr = nc.dram_tensor("xarr", [NTILES, 128, T], bf16, kind="ExternalInput")
    wdup = nc.dram_tensor("wdup", [128, NB * EMB], bf16, kind="ExternalInput")
    bsel = nc.dram_tensor("bsel", [128, 128], bf16, kind="ExternalInput")
    zdev = nc.dram_tensor("zdev", [NTILES, 128, 2 * T], bf16, kind="ExternalOutput")
    with tile.TileContext(nc) as tc, ExitStack() as ctx:
        _build_kernel(ctx, tc, xarr.ap(), wdup.ap(), bsel.ap(), zdev.ap())
    if legalize:
        _legalize_waits(nc)
    return nc


_NC = None


def _get_nc():
    global _NC
    if _NC is None:
        _NC = build_nc()
    return _NC


def _install_ntff_shim():
    """Register the axon NTFF profile hook (dev/testing only; the image's
    antenv package lacks axon_hooks, so bass_utils trace=True would fail)."""
    import sys
    import types

    if "antenv.axon_hooks" in sys.modules:
        return
    mod = types.ModuleType("antenv.axon_hooks")
    mod._hook = None

    def set_axon_ntff_profile_hook(h):
        mod._hook = h

    def get_axon_ntff_profile_hook():
        return mod._hook

    mod.set_axon_ntff_profile_hook = set_axon_ntff_profile_hook
    mod.get_axon_ntff_profile_hook = get_axon_ntff_profile_hook
    sys.modules["antenv.axon_hooks"] = mod
    try:
        import antenv

        antenv.axon_hooks = mod
    except ImportError:
        pass
    try:
        from trn_agent_boot.trn_boot import _ntff_profile_via_ctypes

        mod._hook = _ntff_profile_via_ctypes("/opt/axon/libaxon_pjrt.so")
    except Exception as e:
        print(f"ntff shim: no hook ({e})")


def _host_params(ln_w, ln_b, fc_w, fc_b):
    lw = ln_w.astype(np.float64)
    lb = ln_b.astype(np.float64)
    fw = fc_w.astype(np.float64)
    fb = fc_b.astype(np.float64)
    W1 = lw[:, :, None] * fw                          # [NB, FEAT, EMB]
    Wpp = W1 - W1.sum(1, keepdims=True) / FEAT        # mean-fold
    bp = fb + (lb[:, :, None] * fw).sum(1)            # [NB, EMB]
    # partition q = i*32 + c*16 + w  maps to feat index c*32 + w*2 + i
    q = np.arange(FEAT)
    i = q // 32
    c = (q % 32) // 16
    w = q % 16
    perm = c * 32 + w * 2 + i
    Wq = Wpp[:, perm, :]                              # [NB, 64(q), EMB]
    whalf = np.ascontiguousarray(Wq.transpose(1, 0, 2).reshape(FEAT, NB * EMB))
    wdup_np = np.concatenate([whalf, whalf], axis=0).astype(ml_dtypes.bfloat16)
    bp_np = bp.astype(np.float32)                     # [NB, EMB]
    bsel_np = np.zeros((128, 128), np.float32)
    for i in range(GROUP):
        bsel_np[32 * i, 0:64] = 1.0
        bsel_np[32 * i + 1, 64:128] = 1.0
    bsel_np = bsel_np.astype(ml_dtypes.bfloat16)
    return wdup_np, bp_np, bsel_np


def _pack_x(x_real, x_imag):
    """[B,C,F,T] x2 -> [B, NTILES, 128, T] bf16: partition q = i*32+c*16+w."""
    xr = x_real.reshape(B, C, NTILES, 2, BW, T)   # [b, c, tile, bp, w, t]
    xi = x_imag.reshape(B, C, NTILES, 2, BW, T)
    out = np.empty((B, NTILES, 2, 2, C, BW, T), np.float32)
    out[:, :, :, 0] = xr.transpose(0, 2, 3, 1, 4, 5)
    out[:, :, :, 1] = xi.transpose(0, 2, 3, 1, 4, 5)
    return np.ascontiguousarray(
        out.reshape(B, NTILES, 128, T).astype(ml_dtypes.bfloat16)
    )


def _unpack_z(zdev, bp):
    """[B, NTILES, 128, 2048] bf16 -> [B, NB, T, EMB] f32 (+ bias b'): per
    tile the free axis is (band, t) and partitions are e."""
    nb = zdev.shape[0]
    z = zdev.astype(np.float32).reshape(nb, NTILES, EMB, 2, T)
    z = z.transpose(0, 1, 3, 4, 2)                # [b, tile, band, t, e]
    z = np.ascontiguousarray(z.reshape(nb, NB, T, EMB))
    z += bp[None, :, None, :]
    return z


def kernel(x_real, x_imag, ln_w, ln_b, fc_w, fc_b, _trace=False):
    x_real = np.asarray(x_real, dtype=np.float32)
    x_imag = np.asarray(x_imag, dtype=np.float32)
    ln_w = np.asarray(ln_w, dtype=np.float32)
    ln_b = np.asarray(ln_b, dtype=np.float32)
    fc_w = np.asarray(fc_w, dtype=np.float32)
    fc_b = np.asarray(fc_b, dtype=np.float32)

    if _trace:
        _install_ntff_shim()
    wdup_np, bp_np, bsel_np = _host_params(ln_w, ln_b, fc_w, fc_b)
    xarr = _pack_x(x_real, x_imag)
    nc = _get_nc()
    in_maps = [
        {
            "xarr": xarr[i],
            "wdup": wdup_np,
            "bsel": bsel_np,
        }
        for i in range(NCORES)
    ]
    res = run_bass_kernel_spmd(nc, in_maps, list(range(NCORES)), trace=_trace)
    if _trace and res.exec_time_ns is not None:
        print(f"HW exec time: {res.exec_time_ns} ns")
        if res.instructions_and_trace is not None:
            print(f"trace: {res.instructions_and_trace[1]}")
    zdev = np.stack([res.results[i]["zdev"] for i in range(NCORES)], axis=0)
    return _unpack_z(zdev, bp_np)
